# revision 1
# baseline (speedup 1.0000x reference)
"""Multi-head self-attention with RoPE — Trainium2 Bass/Tile kernel, 8 NeuronCores.

Sharding: batch x head tensor-parallel. Core pair (2b, 2b+1) handles batch b;
within a pair each core computes 8 of the 16 heads (W_q/W_k/W_v column-sharded,
W_o row-sharded), then a per-q-tile pairwise ReduceScatter sums the
output-projection partials (overlapped with the next tile's compute).

Device layout notes:
 - All projections contract d_model on the partition dim; Q/K are produced
   transposed [d_k, seq] per head so attention scores come out transposed
   [k, q] ("S^T" layout): softmax reduction runs across partitions via a
   ones-stationary PE matmul and the AV matmul needs no transposes at all.
 - RoPE is applied via a host-side even/odd permutation of the W_q/W_k rows
   plus [cos;cos] and [sin;-sin] tables; the partition-half swap is done with
   DVE partition-shifted reads; the final add runs on GpSimd to keep DVE free.
 - No max-subtraction in softmax: scores here are bounded (|s| < ~10), exp is
   safe in f32/bf16. Causal masking adds -60 within the 128x128 diagonal
   blocks only; fully-masked columns are skipped via free-dim trimming.
 - Score chunks are computed in pairs into a 2-bank PSUM tile so one Scalar
   activation computes exp for both chunks (halves ACT instruction overhead).
   The den/AV matmuls for pair j are emitted after the scores of pair j+1 so
   the PE never waits on the exp latency.
 - Matmuls run in bf16 with f32 PSUM accumulation; 1/sqrt(d_k) is folded into
   W_q on the host.
"""
import numpy as np
import ml_dtypes

D_MODEL = 2048
N_HEADS = 16
D_K = 128
B = 4
S = 2048
THETA = 10000.0
N_CORES = 8
HPC = N_HEADS // 2     # heads per core
HROWS = HPC * D_K      # 1024 = per-core projection width
NQT = S // 512         # 4 q-tiles of 512
NEG = -60.0
BF16 = ml_dtypes.bfloat16

_cache = {}


def _host_prep(x, token_positions, W_q, W_k, W_v, W_o):
    """Per-core input maps (sharding + layout prep, all host-side numpy)."""
    x = np.asarray(x, np.float32)
    W_q = np.asarray(W_q, np.float32)
    W_k = np.asarray(W_k, np.float32)
    W_v = np.asarray(W_v, np.float32)
    W_o = np.asarray(W_o, np.float32)
    pos = np.asarray(token_positions).astype(np.float32)

    half = D_K // 2
    inv_freq = (THETA ** (-(np.arange(half, dtype=np.float32) * 2.0) / D_K)).astype(np.float32)
    ang = pos[:, None] * inv_freq[None, :]          # [S, 64]
    cos = np.cos(ang).astype(np.float32).T          # [64, S]
    sin = np.sin(ang).astype(np.float32).T
    cos2 = np.concatenate([cos, cos], axis=0).astype(BF16)   # [128, S] bf16
    sin2 = np.concatenate([-sin, sin], axis=0).astype(BF16)  # [128, S] bf16 (pre-swapped)

    perm = np.concatenate([np.arange(0, D_K, 2), np.arange(1, D_K, 2)])

    # 128x128 diagonal-block causal mask: mask[k, c] = 0 if c >= k else NEG
    kl = np.arange(128)[:, None]
    cc = np.arange(128)[None, :]
    mask128 = np.where(kl <= cc, 0.0, NEG).astype(np.float32)  # [128,128]

    in_maps = []
    for c in range(N_CORES):
        b = c // 2
        hh = c % 2
        hsel = slice(hh * HROWS, (hh + 1) * HROWS)

        def permute_heads(Wrows):
            Wr = Wrows.reshape(HPC, D_K, D_MODEL)[:, perm, :]
            return Wr.reshape(HROWS, D_MODEL)

        wq = permute_heads(W_q[hsel]) / np.sqrt(np.float32(D_K))
        wk = permute_heads(W_k[hsel])
        wv = W_v[hsel]
        wo = W_o[:, hsel]                            # [2048, 1024]

        # DMA-optimal pre-tiling: [tile_idx, partition, chunk, cols] so each
        # (tile, partition) source run is contiguous (full-bandwidth DMA).
        xT = x[b].T.astype(BF16)                      # [2048 dm, 2048 rows]
        wqT, wkT, wvT = wq.T.astype(BF16), wk.T.astype(BF16), wv.T.astype(BF16)
        woT = wo.T.astype(BF16)                       # [1024, 2048]
        in_maps.append({
            "x_t": np.ascontiguousarray(
                xT.reshape(16, 128, 4, 512).transpose(2, 1, 0, 3)),   # [4,128,16,512]
            "wq_t": np.ascontiguousarray(
                wqT.reshape(16, 128, 8, 128).transpose(2, 1, 0, 3)),  # [8,128,16,128]
            "wk_t": np.ascontiguousarray(
                wkT.reshape(16, 128, 8, 128).transpose(2, 1, 0, 3)),  # [8,128,16,128]
            "wv_t": np.ascontiguousarray(
                wvT.reshape(16, 128, 2, 512).transpose(2, 1, 0, 3)),  # [2,128,16,512]
            "wo_t": np.ascontiguousarray(
                woT.reshape(8, 128, 4, 512).transpose(2, 1, 0, 3)),   # [4,128,8,512]
            "cos2": cos2,
            "sin2": sin2,
            "mask128": mask128,
        })
    return in_maps


def _build_program(use_collective=True):
    import concourse.bass as bass
    import concourse.mybir as mybir
    import concourse.tile as tile
    from concourse import bacc, bass_isa

    f32 = mybir.dt.float32
    bf16 = mybir.dt.bfloat16
    EXP = mybir.ActivationFunctionType.Exp
    COPY = mybir.ActivationFunctionType.Copy
    MUL = mybir.AluOpType.mult
    ADD = mybir.AluOpType.add

    nc = bacc.Bacc("TRN2", target_bir_lowering=False, debug=False,
                   num_devices=N_CORES)

    x_td = nc.dram_tensor("x_t", [4, 128, 16, 512], bf16, kind="ExternalInput")
    wq_td = nc.dram_tensor("wq_t", [8, 128, 16, 128], bf16, kind="ExternalInput")
    wk_td = nc.dram_tensor("wk_t", [8, 128, 16, 128], bf16, kind="ExternalInput")
    wv_td = nc.dram_tensor("wv_t", [2, 128, 16, 512], bf16, kind="ExternalInput")
    wo_td = nc.dram_tensor("wo_t", [4, 128, 8, 512], bf16, kind="ExternalInput")
    cos2_d = nc.dram_tensor("cos2", [128, S], bf16, kind="ExternalInput")
    sin2_d = nc.dram_tensor("sin2", [128, S], bf16, kind="ExternalInput")
    mask_d = nc.dram_tensor("mask128", [128, 128], f32, kind="ExternalInput")
    out_d = nc.dram_tensor("out", [S // 2 if use_collective else S, D_MODEL],
                           bf16 if use_collective else f32, kind="ExternalOutput")

    DM_CH = D_MODEL // 128  # 16 contraction chunks

    with tile.TileContext(nc) as tc:
        with (
            tc.tile_pool(name="const", bufs=1) as cpool,
            tc.tile_pool(name="big", bufs=1) as bigpool,
            tc.tile_pool(name="xs", bufs=2) as xpool,
            tc.tile_pool(name="w", bufs=2) as wpool,
            tc.tile_pool(name="wv", bufs=2) as wvpool,
            tc.tile_pool(name="qt", bufs=2) as qpool,
            tc.tile_pool(name="tmp", bufs=2) as tpool,
            tc.tile_pool(name="den", bufs=2) as dpool,
            tc.tile_pool(name="p", bufs=3) as ppool,
            tc.tile_pool(name="osb", bufs=2) as opool,
            tc.tile_pool(name="psP", bufs=2, space="PSUM") as psP,
            tc.tile_pool(name="psS", bufs=2, space="PSUM") as psS,
            tc.tile_pool(name="psC", bufs=2, space="PSUM") as psC,
            tc.tile_pool(name="dram", bufs=1, space="DRAM") as dram,
        ):
            # ---- constants ----
            mask128 = cpool.tile([128, 128], f32, tag="mask128")
            nc.gpsimd.dma_start(mask128[:], mask_d[:])
            # full-width all-ones stationary: the den matmul then produces the
            # softmax denominator replicated across all 128 partitions (no
            # partition_broadcast needed) and stays a full-array matmul (a
            # [1,512] col_grp=q0 output costs ~+200ns of PE pipeline breakage
            # per den matmul).
            ones = cpool.tile([128, 128], bf16, tag="ones")
            nc.gpsimd.memset(ones[:], 1.0)

            # ---- persistent phase-A outputs ----
            kTr = bigpool.tile([128, HPC, S], bf16, tag="kTr")      # [dk, h, keys]
            v_sb = bigpool.tile([128, S // 128, HROWS], bf16, tag="v")  # [row, kc, hdim]

            # DRAM bounce buffers for the per-tile collectives
            pout = dram.tile([S, D_MODEL], bf16 if use_collective else f32,
                             tag="pout")
            rs_out = dram.tile([NQT, 512 // 2, D_MODEL], bf16, tag="rs_out")

            def rope_epilogue(ps, out_ap, cos_t, sin_t):
                """out = ps*cos2 + swap(ps)*sin2sw, cast bf16. ps: [128,512] psum.
                sin2 is host-pre-swapped ([-sin; sin]); the partition-half swap
                of ps happens via DVE partition-shifted reads (HW-verified).
                Final add runs on GpSimd so DVE stays available."""
                u = tpool.tile([128, 512], f32, tag="u", bufs=1)
                t = tpool.tile([128, 512], f32, tag="t")
                nc.vector.tensor_tensor(t[:], ps[:], cos_t[:], MUL)
                nc.vector.tensor_tensor(u[0:64, :], ps[64:128, :],
                                        sin_t[0:64, :], MUL)
                nc.vector.tensor_tensor(u[64:128, :], ps[0:64, :],
                                        sin_t[64:128, :], MUL)
                nc.gpsimd.tensor_tensor(out_ap, t[:], u[:], ADD)

            # ---- per q-tile pipeline ----
            # The O projection of tile qt-1 is emitted BETWEEN the projections
            # of tile qt and the attention of tile qt. Its ReduceScatter is
            # triggered right after it: the RS input (pout) completes at the
            # end of O(qt-1), which on the PE FIFO is immediately before
            # attention(qt) — so the collective physically runs during
            # attention(qt), the only DMA-quiet phase. (An in-flight
            # collective slows concurrent DMA transfers ~5x; over the
            # projection phases that starves weight prefetch and re-throttles
            # HAM.)
            def emit_o_phase(oqt, o_ctx, o_wos0):
                for half in range(2):
                    if half == 0:
                        wos = o_wos0
                    else:
                        # second weight pair streams on the gpsimd queue: its
                        # slot wait (for the first pair's reads) must not
                        # block the sync queue's prefetch DMAs
                        wos = []
                        for sub in range(2):
                            wo = wpool.tile([128, HPC, 512], bf16, tag="wo",
                                            name=f"wo_{oqt}_1_{sub}")
                            for wc in range(2):
                                nc.gpsimd.dma_start(
                                    wo[:, 4 * wc:4 * wc + 4, :],
                                    wo_td[2 + sub, :, 4 * wc:4 * wc + 4, :])
                            wos.append(wo)
                    for rc in range(4):
                        o2 = psS.tile([128, 2, 512], f32, tag="s2",
                                      name=f"o2_{oqt}_{half}_{rc}")
                        for hh in range(HPC):
                            for sub in range(2):
                                nc.tensor.matmul(
                                    o2[:, sub, :],
                                    o_ctx[:, hh, rc * 128:(rc + 1) * 128],
                                    wos[sub][:, hh, :],
                                    start=(hh == 0), stop=(hh == HPC - 1))
                        osb = opool.tile([128, 2, 512],
                                         bf16 if use_collective else f32,
                                         tag="osb")
                        nc.scalar.activation(osb[:], o2[:], COPY)
                        r0 = oqt * 512 + rc * 128
                        nc.gpsimd.dma_start(
                            pout[r0:r0 + 128, half * 1024:(half + 1) * 1024],
                            osb[:])
                if use_collective:
                    nc.gpsimd.collective_compute(
                        "ReduceScatter",
                        mybir.AluOpType.add,
                        replica_groups=[[0, 1], [2, 3], [4, 5], [6, 7]],
                        ins=[pout[oqt * 512:(oqt + 1) * 512, :].opt()],
                        outs=[rs_out[oqt].opt()],
                    )

            for qt in range(NQT):
                qs = slice(qt * 512, (qt + 1) * 512)
                # First q-tile: the m=0 weight DMA goes first so the very
                # first matmul only waits for one small weight tile plus one
                # xs sub-DMA (fast time-to-first-matmul).
                wt0 = None
                if qt == 0:
                    wt0 = wpool.tile([128, DM_CH, 128], bf16, tag="w",
                                     name="wt_first", bufs=4)
                    nc.sync.dma_start(wt0[:], wq_td[0])
                xs = xpool.tile([128, DM_CH, 512], bf16, tag="xs")
                for xc in range(4):
                    eng = nc.sync if xc < 2 else nc.scalar
                    eng.dma_start(xs[:, 4 * xc:4 * xc + 4, :],
                                  x_td[qt, :, 4 * xc:4 * xc + 4, :])
                cos_t = tpool.tile([128, 512], bf16, tag="cos")
                sin_t = tpool.tile([128, 512], bf16, tag="sin")
                nc.sync.dma_start(cos_t[:], cos2_d[:, qs])
                nc.sync.dma_start(sin_t[:], sin2_d[:, qs])

                # Q projection for this q-tile (transposed + RoPE)
                qTr = qpool.tile([128, HPC, 512], bf16, tag="qTr")
                for m in range(HPC):
                    if wt0 is not None and m == 0:
                        wt = wt0
                    else:
                        wt = wpool.tile([128, DM_CH, 128], bf16, tag="w",
                                        bufs=4)
                        for wc in range(2):
                            nc.sync.dma_start(
                                wt[:, 8 * wc:8 * wc + 8, :],
                                wq_td[m, :, 8 * wc:8 * wc + 8, :])
                    ps = psP.tile([128, 512], f32, tag="ps")
                    for k in range(DM_CH):
                        nc.tensor.matmul(ps[:], wt[:, k, :], xs[:, k, :],
                                         start=(k == 0), stop=(k == DM_CH - 1))
                    rope_epilogue(ps, qTr[:, m, :], cos_t, sin_t)

                # K projection for key rows of this slice (transposed + RoPE)
                for m in range(HPC):
                    wt = wpool.tile([128, DM_CH, 128], bf16, tag="w",
                                    bufs=4)
                    for wc in range(2):
                        nc.sync.dma_start(
                            wt[:, 8 * wc:8 * wc + 8, :],
                            wk_td[m, :, 8 * wc:8 * wc + 8, :])
                    ps = psP.tile([128, 512], f32, tag="ps")
                    for k in range(DM_CH):
                        nc.tensor.matmul(ps[:], wt[:, k, :], xs[:, k, :],
                                         start=(k == 0), stop=(k == DM_CH - 1))
                    rope_epilogue(ps, kTr[:, m, qs], cos_t, sin_t)

                # V projection for key rows of this slice (natural layout).
                # nv is the INNER loop so each xs-chunk stationary serves two
                # matmuls (one LDWEIGHTS per pair) into a 2-bank PSUM tile.
                wvt = []
                for nv in range(2):
                    wvq = wvpool.tile([128, DM_CH, 512], bf16, tag="wv",
                                      name=f"wv_{qt}_{nv}")
                    for wc in range(2):
                        nc.scalar.dma_start(
                            wvq[:, 8 * wc:8 * wc + 8, :],
                            wv_td[nv, :, 8 * wc:8 * wc + 8, :])
                    wvt.append(wvq)
                for rc in range(4):
                    v2 = psS.tile([128, 2, 512], f32, tag="s2",
                                  name=f"v2_{qt}_{rc}")
                    for k in range(DM_CH):
                        for nv in range(2):
                            nc.tensor.matmul(
                                v2[:, nv, :], xs[:, k, rc * 128:(rc + 1) * 128],
                                wvt[nv][:, k, :],
                                start=(k == 0), stop=(k == DM_CH - 1))
                    nc.vector.tensor_copy(v_sb[:, qt * 4 + rc, :], v2[:])

                # attention for this q-tile (S^T layout, PE denominator).
                # Score chunks run in pairs into a 2-bank PSUM tile; den/AV of
                # pair j are emitted after the scores+exp of pair j+1 so the
                # exp latency is hidden behind PE work.
                ctx_t = qpool.tile([128, HPC, 512], bf16, tag="ctx", bufs=1)
                # prefetch the first O-projection weight pair on the sync
                # queue (idle during attention) so the O phase starts hot
                wos0 = []
                for sub in range(2):
                    wo = wpool.tile([128, HPC, 512], bf16, tag="wo",
                                    name=f"wo_{qt}_0_{sub}")
                    for wc in range(2):
                        nc.sync.dma_start(
                            wo[:, 4 * wc:4 * wc + 4, :],
                            wo_td[sub, :, 4 * wc:4 * wc + 4, :])
                    wos0.append(wo)
                nkc = 4 * (qt + 1)
                npair = nkc // 2

                def emit_denav(p2, j, dh, d_ps, c_ps):
                    for t_ in range(2):
                        kc = 2 * j + t_
                        lo = max(kc - 4 * qt, 0) * 128
                        nc.tensor.matmul(
                            d_ps[:, lo:512], ones[:], p2[:, t_, lo:512],
                            start=(kc == 0), stop=(kc == nkc - 1))
                        nc.tensor.matmul(
                            c_ps[:, lo:512],
                            v_sb[:, kc, dh * 128:(dh + 1) * 128],
                            p2[:, t_, lo:512],
                            start=(kc == 0), stop=(kc == nkc - 1))

                def emit_softmax_tail(dh, d_ps, c_ps):
                    rcpb = dpool.tile([128, 512], f32, tag="rcpb", bufs=1)
                    nc.vector.reciprocal_approx_fast(rcpb[:], d_ps[:])
                    nc.vector.tensor_tensor(ctx_t[:, dh, :], c_ps[:],
                                            rcpb[:], MUL)

                # pend slides ACROSS head boundaries: den/AV of pair j are
                # emitted after the scores+exp of the NEXT pair (even into
                # the next head), so the PE never waits on exp latency at
                # head starts. pend = (p2, j, h, den_ps, ctx_ps, is_last).
                pend = None
                for h in range(HPC):
                    ctx_ps = psC.tile([128, 512], f32, tag="ctx")
                    den_ps = psP.tile([128, 512], f32, tag="ps")
                    for j in range(npair):
                        s2 = psS.tile([128, 2, 512], f32, tag="s2")
                        p2 = ppool.tile([128, 2, 512], bf16, tag="p2")
                        los = []
                        for t_ in range(2):
                            kc = 2 * j + t_
                            d = kc - 4 * qt
                            lo = max(d, 0) * 128
                            los.append(lo)
                            nc.tensor.matmul(
                                s2[:, t_, lo:512],
                                kTr[:, h, kc * 128:(kc + 1) * 128],
                                qTr[:, h, lo:512], start=True, stop=True)
                            if d >= 0:
                                nc.vector.tensor_tensor(
                                    s2[:, t_, lo:lo + 128],
                                    s2[:, t_, lo:lo + 128], mask128[:], ADD)
                        if los[0] == los[1]:
                            # one exp over both banks (contiguous PSUM)
                            nc.scalar.activation(p2[:, :, los[0]:512],
                                                 s2[:, :, los[0]:512], EXP)
                        else:
                            for t_ in range(2):
                                nc.scalar.activation(p2[:, t_, los[t_]:512],
                                                     s2[:, t_, los[t_]:512], EXP)
                        if pend is not None:
                            pp2, pj, ph, pd, pc, plast = pend
                            emit_denav(pp2, pj, ph, pd, pc)
                            if plast:
                                emit_softmax_tail(ph, pd, pc)
                        pend = (p2, j, h, den_ps, ctx_ps, j == npair - 1)
                # flush the final pair of the tile's last head
                pp2, pj, ph, pd, pc, _ = pend
                emit_denav(pp2, pj, ph, pd, pc)
                emit_softmax_tail(ph, pd, pc)

                # O projection + this tile's reduce-scatter; the RS overlaps
                # the next tile's projection phases (weight prefetch rides it
                # via the deepened wq/wk ring + split DMAs).
                emit_o_phase(qt, ctx_t, wos0)

                # drain the PREVIOUS tile's RS result (completed during this
                # tile's projections) on the scalar queue
                if use_collective and qt > 0:
                    pq = qt - 1
                    nc.scalar.dma_start(out_d[pq * 256:(pq + 1) * 256, :],
                                        rs_out[pq])

            if use_collective:
                pq = NQT - 1
                nc.scalar.dma_start(out_d[pq * 256:(pq + 1) * 256, :],
                                    rs_out[pq])
            else:
                nc.sync.dma_start(out_d[:], pout[:])

    nc.compile()
    return nc


def kernel(x, token_positions, W_q, W_k, W_v, W_o):
    from concourse.bass_utils import run_bass_kernel_spmd

    if "nc" not in _cache:
        _cache["nc"] = _build_program()
    nc = _cache["nc"]

    in_maps = _host_prep(x, token_positions, W_q, W_k, W_v, W_o)
    res = run_bass_kernel_spmd(nc, in_maps, list(range(N_CORES)))
    return assemble([res.results[c]["out"] for c in range(N_CORES)])


def assemble(outs):
    """Stitch per-core [1024, 2048] outputs into [B, S, D_MODEL].

    Each per-q-tile ReduceScatter splits that tile's 512 rows between the
    pair: core 2b holds rows [qt*512, qt*512+256), core 2b+1 holds rows
    [qt*512+256, (qt+1)*512). Outputs arrive bf16."""
    out = np.empty((B, S, D_MODEL), np.float32)
    for b in range(B):
        ev = outs[2 * b].astype(np.float32)
        od = outs[2 * b + 1].astype(np.float32)
        for qt in range(NQT):
            out[b, qt * 512:qt * 512 + 256] = ev[qt * 256:(qt + 1) * 256]
            out[b, qt * 512 + 256:(qt + 1) * 512] = od[qt * 256:(qt + 1) * 256]
    return out

